# revision 1
# baseline (speedup 1.0000x reference)
"""Gaussian kernel matrix on 8 Trainium2 NeuronCores — circulant-symmetric fp8.

out = exp(-d2 / (2*sigma^2)),  d2[i,j] = ||x_i||^2 + ||x_j||^2 - 2 x_i.x_j,
sigma^2 = mean(d2) = 2*mean(sq) - 2*||mean(X, axis=0)||^2.

Strategy:
- Symmetry: core c computes rows [c*512,(c+1)*512) x a wrapped column window
  of 2560 cols starting at c*512 (5 of 8 j-blocks). Every unordered (i,j)
  pair is covered by at least one core; the host mirrors the remaining
  blocks by transposition. 0.625x compute/output vs full slabs.
- GEMM in fp8 e4m3 with DoubleRow (K=256 per matmul): X quantized on host;
  the kernel then computes the EXACT Gaussian kernel of the quantized
  points (simulated fro err ~2.5e-3 incl bf16 output, gate is 2e-2).
- Epilogue fused into one ACT pass: PSUM accumulates
  G = x_i.x_j + q_j, q_j = -(sq_j-512)/2 added by a K=1 fp8 aug matmul
  (mean-centered so fp8 suffices), then out = Exp(scale*G + bias_i) with
  scale = 1/sigma^2, bias_i = -(sq_i+512)/(2 sigma^2), written straight
  to bf16 and DMA'd out. No vector-engine multiply over the output.
- Stats: window x^2 on DVE(kt0)+GpSimd(kt1) in fp8, sq via DoubleRow
  ones-matmuls. sigma^2 is estimated from the first 1024 window columns
  (mean(d2) ~ 2*mean(sq); the ||mean X||^2 term is 0.012% and the sample
  SE ~0.2% -- both far under the fp8 noise floor and the 2e-2 gate). An
  exact AllReduce was measured at ~53us of firmware latency on this
  fabric, completely serializing the kernel, so no collectives.
"""
import numpy as np
import sys

sys.path.insert(0, "/opt/trn_rl_repo")
from concourse import bass, tile, mybir  # noqa: E402
from concourse.bass_utils import run_bass_kernel_spmd  # noqa: E402

N, D, NCORES = 4096, 512, 8
RPC = 512                  # output rows per core
P = 128                    # partitions
KT = 2                     # DoubleRow k-tiles (256 contraction rows each)
JB = 512                   # j-block width
W = 5                      # window j-blocks per core
WIN = W * JB               # 2560 window columns
NT = RPC // P              # 4 row-tiles per core
f32 = mybir.dt.float32
bf16 = mybir.dt.bfloat16
fp8 = mybir.dt.float8e4
ACTF = mybir.ActivationFunctionType
ALU = mybir.AluOpType
DR = mybir.MatmulPerfMode.DoubleRow
DRS = mybir.MatmulPerfMode.DoubleRowSwInterleave


def _split_waits(nc, max_waits=1):
    """walrus in this image encodes at most one sync-wait per instruction;
    split extras into single-wait NOPs placed just before the instruction."""
    for fn in nc.m.functions:
        for bb in fn.blocks:
            out = []
            for inst in bb.instructions:
                si = inst.sync_info
                if si and si.on_wait and len(si.on_wait) > max_waits:
                    waits = list(si.on_wait)
                    extra, keep = waits[:-max_waits], waits[-max_waits:]
                    for j, w in enumerate(extra):
                        out.append(mybir.InstNoOp(
                            name=f"{inst.name}-ws{j}", engine=inst.engine,
                            sync_info=mybir.SyncInfo(on_wait=[w], on_update=[])))
                    si.on_wait = keep
                out.append(inst)
            bb.instructions = out


def build():
    nc = bass.Bass()
    wj_in = nc.dram_tensor("wj", [KT, P, 2, WIN], fp8, kind="ExternalInput")
    xtc_in = nc.dram_tensor("xtc", [KT, P, RPC, 2], fp8, kind="ExternalInput")
    out_d = nc.dram_tensor("out", [RPC, WIN], bf16, kind="ExternalOutput")

    with tile.TileContext(nc) as tc:
        with (
            tc.tile_pool(name="xt", bufs=1) as xt_pool,
            tc.tile_pool(name="x2", bufs=1) as x2_pool,
            tc.tile_pool(name="ot", bufs=1) as ot_pool,
            tc.tile_pool(name="small", bufs=1) as small_pool,
            tc.tile_pool(name="g", bufs=7, space="PSUM") as g_pool,
            tc.tile_pool(name="aux", bufs=1, space="PSUM") as aux_pool,
        ):
            # ---- constants ------------------------------------------------
            ones_f = small_pool.tile([1, P], f32, tag="ones_f")
            nc.vector.memset(ones_f[:], 1.0)
            # DoubleRow weights need plane step %16 == 0 -> pad to [P, 2, 16]
            o8d_f = small_pool.tile([P, 2, 16], f32, tag="o8d_f")
            nc.vector.memset(o8d_f[:], 1.0)
            ones8dr_t = small_pool.tile([P, 2, 16], fp8, tag="ones8dr")
            nc.vector.tensor_copy(ones8dr_t[:], o8d_f[:])
            ones8dr = ones8dr_t[:, :, 0:1]

            # Dummy Square/Exp activations eat the one-time ACT table loads
            # (~1.2us each) during the DMA dead-time at kernel start.
            o8r_f = small_pool.tile([1, 16], f32, tag="o8r_f")
            nc.vector.memset(o8r_f[:], 1.0)
            warm_sb = small_pool.tile([1, 16], f32, tag="warm_sb")
            nc.scalar.activation(warm_sb[:], o8r_f[:], ACTF.Square)
            nc.scalar.activation(warm_sb[:], o8r_f[:], ACTF.Exp)

            # ---- input DMA: w-chunked, spread over idle engine queues -----
            wj = [xt_pool.tile([P, 2, WIN], fp8, name=f"wj{k}", tag=f"wj{k}")
                  for k in range(KT)]
            # weights in DoubleRowSwInterleave layout [p, m, 2] (pairs
            # adjacent, m reversed per 128-slice): contiguous weight reads
            xtc = [xt_pool.tile([P, RPC, 2], fp8, name=f"xtc{k}", tag=f"xtc{k}")
                   for k in range(KT)]
            nc.gpsimd.dma_start(xtc[0][:], xtc_in[0])
            nc.gpsimd.dma_start(xtc[1][:], xtc_in[1])
            for w in range(W):
                sl = slice(w * JB, (w + 1) * JB)
                nc.sync.dma_start(wj[0][:, :, sl], wj_in[0, :, :, sl])
                nc.sync.dma_start(wj[1][:, :, sl], wj_in[1, :, :, sl])

            # ---- stats tiles ---------------------------------------------
            x2 = [x2_pool.tile([P, 2, WIN], fp8, name=f"x2_{k}", tag=f"x2_{k}")
                  for k in range(KT)]
            acc = small_pool.tile([1, 1], f32, tag="acc")  # sigma sample acc

            # x^2 per (w, kt) chunk. The urgent w0 chunks (gating sigma/q0
            # and every drain) run in parallel on DVE+GpSimd right after the
            # first DMA chunks land; ACT (1 elem/cycle Square) takes the
            # w1/w2 bulk, DVE/GpSimd split the late w3/w4.
            X2_ENG = {(0, 0): "dve", (0, 1): "gps", (1, 0): "act",
                      (1, 1): "act", (2, 0): "act", (2, 1): "act",
                      (3, 0): "dve", (3, 1): "gps", (4, 0): "dve",
                      (4, 1): "gps"}

            def emit_x2(w):
                sl = slice(w * JB, (w + 1) * JB)
                for k in range(KT):
                    eng = X2_ENG[(w, k)]
                    if eng == "act":
                        nc.scalar.activation(x2[k][:, :, sl], wj[k][:, :, sl],
                                             ACTF.Square)
                    elif eng == "dve":
                        nc.vector.tensor_mul(x2[k][:, :, sl], wj[k][:, :, sl],
                                             wj[k][:, :, sl])
                    else:
                        nc.gpsimd.tensor_mul(x2[k][:, :, sl], wj[k][:, :, sl],
                                             wj[k][:, :, sl])

            # sq(w) = ones^T x^2 (DoubleRow ones-matmul into aux psum),
            # then q_w = -(sq_w - 512)/2 in fp8 written straight into the
            # sacrificed contraction row of wj[1] (partition 96, plane 1 =
            # logical dim 480, whose weight is 1): the column term then rides
            # the second main matmul for free — no separate aug matmul.
            # The q op runs on DVE or GpSimd, whichever queue is free then.
            Q_ENG = {0: "gps", 1: "dve", 2: "dve", 3: "gps", 4: "gps"}

            def emit_sq_mm(w):
                sl = slice(w * JB, (w + 1) * JB)
                sqp = aux_pool.tile([1, JB], f32, name=f"sqp{w}", tag="aux")
                for k in range(KT):
                    nc.tensor.matmul(sqp[:], ones8dr, x2[k][:, :, sl],
                                     start=(k == 0), stop=(k == KT - 1),
                                     perf_mode=DR)
                nc.vector.tensor_scalar(wj[1][96:97, 1, sl], sqp[:],
                                        512.0, -0.5,
                                        ALU.subtract, ALU.mult)
                return sqp

            with tc.high_priority():
                emit_x2(0)
                sl0 = slice(0, JB)
                sqp0 = aux_pool.tile([1, JB], f32, name="sqp0", tag="aux")
                for k in range(KT):
                    nc.tensor.matmul(sqp0[:], ones8dr, x2[k][:, :, sl0],
                                     start=(k == 0), stop=(k == KT - 1),
                                     perf_mode=DR)
                # sigma^2 ~ 2*mean(sq over own 512 cols) = sum/256
                nc.vector.tensor_reduce(acc[:], sqp0[:],
                                        axis=mybir.AxisListType.X,
                                        op=ALU.add)
                nc.vector.tensor_scalar(wj[1][96:97, 1, sl0], sqp0[:],
                                        512.0, -0.5,
                                        ALU.subtract, ALU.mult)
                sig = small_pool.tile([1, 1], f32, tag="sig")
                nc.vector.tensor_scalar_mul(sig[:], acc[:], 1.0 / 256.0)
                r = small_pool.tile([1, 1], f32, tag="r")
                nc.vector.reciprocal(r[:], sig[:])        # 1/sigma^2
                mhr = small_pool.tile([1, 1], f32, tag="mhr")
                nc.vector.tensor_scalar_mul(mhr[:], r[:], -0.5)
                emit_x2(1)
                emit_sq_mm(1)
            scale_col = small_pool.tile([P, 1], f32, tag="scale_col")
            mhr_col = small_pool.tile([P, 1], f32, tag="mhr_col")

            # ---- main GEMM + fused epilogue -------------------------------
            def tile_mms(gp, t, w):
                for k in range(KT):
                    nc.tensor.matmul(
                        gp[:], xtc[k][:, t * P:(t + 1) * P, :],
                        wj[k][:, :, w * JB:(w + 1) * JB],
                        start=(k == 0), stop=(k == KT - 1), perf_mode=DRS)

            bias_col = small_pool.tile([P, NT], f32, tag="bias_col")

            def emit_sigma_stage():
                # Emitted after tile (0,0)'s matmuls, all high-priority:
                # sq_own directly in partition layout via tiny DoubleRow
                # matmuls (x^2 own-block slices as weights, ones moving),
                # r/mhr broadcasts, then bias in one per-partition-scalar
                # DVE op: bias_col = (sq_own + 512) * (-1/(2 sigma^2)).
                with tc.high_priority():
                    bp = g_pool.tile([P, NT], f32, name="bp", tag="g")
                    for tt in range(NT):
                        for k in range(KT):
                            nc.tensor.matmul(
                                bp[:, tt:tt + 1],
                                x2[k][:, :, tt * P:(tt + 1) * P],
                                ones8dr,
                                start=(k == 0), stop=(k == KT - 1),
                                perf_mode=DR)
                    for val, col in ((r, scale_col), (mhr, mhr_col)):
                        pb = aux_pool.tile([P, 1], f32,
                                           name=f"pb_{col.tensor.name}",
                                           tag="aux")
                        nc.tensor.matmul(pb[:], ones_f[:], val[:],
                                         start=True, stop=True)
                        nc.vector.tensor_copy(col[:], pb[:])
                    nc.vector.tensor_scalar(bias_col[:], bp[:], 512.0,
                                            mhr_col[:, 0:1],
                                            ALU.add, ALU.mult)

            def drain(gp, t, w):
                ot = ot_pool.tile([P, JB], bf16,
                                  name=f"ot_{t}_{w}", tag="ot", bufs=4)
                nc.scalar.activation(ot[:], gp[:], ACTF.Exp,
                                     bias=bias_col[:, t:t + 1],
                                     scale=scale_col[:, 0:1])
                nc.sync.dma_start(
                    out_d[t * P:(t + 1) * P, w * JB:(w + 1) * JB], ot[:])

            # sigma stage sits after tile (0,0)'s matmuls so scale_col /
            # bias_col are written (in program order) before their first
            # reader — the framework derives dependencies from program order.
            for w in range(W):
                for t in range(NT):
                    gp = g_pool.tile([P, JB], f32, name=f"gp_{t}_{w}",
                                     tag="g")
                    tile_mms(gp, t, w)
                    if (w, t) == (0, 0):
                        emit_sigma_stage()
                    elif w == 0 and t > 0:
                        emit_x2(t + 1)
                        emit_sq_mm(t + 1)
                    drain(gp, t, w)

    _split_waits(nc)
    return nc


_NC = None


def _dr_layout(a):
    """[512, M] -> [KT, P, 2, M] DoubleRow plane layout (d = kt*256+i*128+p)."""
    return np.ascontiguousarray(a.reshape(KT, 2, P, a.shape[1])
                                .transpose(0, 2, 1, 3))


def _swi_layout(a):
    """[512, M] -> [KT, P, M, 2] DoubleRowSwInterleave weights: per k-tile,
    (plane0, plane1) pairs adjacent along the last axis, with the m index
    reversed inside each 128-wide stationary slice. Logical dim 480
    (kt=1, p=96, plane=1) is the augmentation row: its weight is 1 and the
    device writes q_j into the matching rhs slot."""
    m = a.shape[1]
    w = a.reshape(KT, 2, P, m).transpose(0, 2, 3, 1)    # [kt, p, m, i]
    w = w.reshape(KT, P, m // P, P, 2)[:, :, :, ::-1, :]
    w = np.ascontiguousarray(w.reshape(KT, P, m, 2))
    w[1, 96, :, 1] = 1.0
    return w


def make_in_maps(X):
    import ml_dtypes
    X8 = np.asarray(X, dtype=ml_dtypes.float8_e4m3)
    XT8 = np.ascontiguousarray(X8.T)              # [512, 4096]
    maps = []
    for c in range(NCORES):
        lo = c * RPC
        win = np.concatenate([XT8[:, lo:], XT8[:, :lo]], axis=1)[:, :WIN]
        maps.append({
            "wj": _dr_layout(win),
            "xtc": _swi_layout(XT8[:, lo:lo + RPC]),
        })
    return maps


def assemble(slabs):
    """slabs: per-core [RPC, WIN] (bf16) -> full [N, N] f32 via symmetry."""
    out = np.empty((N, N), dtype=np.float32)
    for c in range(NCORES):
        lo = c * RPC
        slab = np.asarray(slabs[c], dtype=np.float32)
        n1 = min(WIN, N - lo)
        out[lo:lo + RPC, lo:lo + n1] = slab[:, :n1]
        if n1 < WIN:
            out[lo:lo + RPC, :WIN - n1] = slab[:, n1:]
    # mirror block-distance {5,6,7} from their transposed {3,2,1} partners
    for bi in range(NCORES):
        for dd in (5, 6, 7):
            bj = (bi + dd) % NCORES
            out[bi * RPC:(bi + 1) * RPC, bj * RPC:(bj + 1) * RPC] = \
                out[bj * RPC:(bj + 1) * RPC, bi * RPC:(bi + 1) * RPC].T
    return out


def kernel(X: np.ndarray) -> np.ndarray:
    global _NC
    if _NC is None:
        _NC = build()
    res = run_bass_kernel_spmd(_NC, make_in_maps(X),
                               list(range(NCORES))).results
    return assemble([res[c]["out"] for c in range(NCORES)])



# revision 6
# speedup vs baseline: 1.2303x; 1.2303x over previous
"""Gaussian kernel matrix on 8 Trainium2 NeuronCores — host-stats fp8 GEMM.

out = exp(-d2 / (2*sigma^2)),  d2[i,j] = ||x_i||^2 + ||x_j||^2 - 2 x_i.x_j,
sigma^2 = mean(d2) = 2*(mean(sq) - ||mean(X)||^2).

Strategy v2:
- Symmetry: core c computes rows [c*512,(c+1)*512) x a wrapped column window
  of 2560 cols starting at c*512 (5 of 8 j-blocks). Every unordered (i,j)
  pair is covered by at least one core; the host mirrors the remaining
  blocks by transposition. 0.625x compute/output vs full slabs.
- GEMM in fp8 e4m3 with DoubleRowSwInterleave (K=256 per matmul): X
  quantized on host; the kernel computes the EXACT Gaussian kernel of the
  quantized points (fro err ~3e-3 incl bf16 output, gate is 2e-2).
- ALL statistics on the host (O(N*D), trivial next to the O(N^2*D) GEMM):
  sq_j, sigma^2 (exact, from unquantized X), q_j = -(sq_j-512)/2 written
  into the sacrificed contraction row (logical dim 480) of wj so the
  column term rides the main matmul; bias_i/scale shipped as a tiny [P,5]
  f32 input consumed directly by the fused ACT epilogue
  out = Exp(scale*G + bias_i) -> bf16 -> DMA. The device runs NOTHING but
  DMA + matmul + activation.
- PE p-state: the Tensor engine only reaches max clock after ~3us of
  continuous busy and drops back on idle gaps. Dummy warmup matmuls on a
  zeroed scratch tile start the ramp during the input-DMA dead time and
  bridge until the first wj chunk lands.
- ACT overhead amortized by pairing j-blocks: PSUM tiles span 2 banks
  [128,1024] and one Exp pass covers both, halving the ~250ns fixed
  PSUM-access cost per instruction.
- DMA spread across sync/gpsimd/vector queues (inputs and outputs) so no
  single queue serializes; outputs rotate across the three.
"""
import numpy as np
import sys

sys.path.insert(0, "/opt/trn_rl_repo")
from concourse import bass, tile, mybir  # noqa: E402
from concourse.bass_utils import run_bass_kernel_spmd  # noqa: E402

N, D, NCORES = 4096, 512, 8
RPC = 512                  # output rows per core
P = 128                    # partitions
KT = 2                     # DoubleRow k-tiles (256 contraction rows each)
JB = 512                   # j-block width
W = 5                      # window j-blocks per core
WIN = W * JB               # 2560 window columns
NT = RPC // P              # 4 row-tiles per core
NWARM = 24                 # PE warmup matmuls (~150ns each)
f32 = mybir.dt.float32
bf16 = mybir.dt.bfloat16
fp8 = mybir.dt.float8e4
ACTF = mybir.ActivationFunctionType
DRS = mybir.MatmulPerfMode.DoubleRowSwInterleave

# (window col offset, width) groups: pair j-blocks for wide ACT passes
GROUPS = [(0, 1024), (1024, 1024), (2048, 512)]


def _split_waits(nc, max_waits=1):
    """walrus in this image encodes at most one sync-wait per instruction;
    split extras into single-wait NOPs placed just before the instruction."""
    for fn in nc.m.functions:
        for bb in fn.blocks:
            out = []
            for inst in bb.instructions:
                si = inst.sync_info
                if si and si.on_wait and len(si.on_wait) > max_waits:
                    waits = list(si.on_wait)
                    extra, keep = waits[:-max_waits], waits[-max_waits:]
                    for j, w in enumerate(extra):
                        out.append(mybir.InstNoOp(
                            name=f"{inst.name}-ws{j}", engine=inst.engine,
                            sync_info=mybir.SyncInfo(on_wait=[w], on_update=[])))
                    si.on_wait = keep
                out.append(inst)
            bb.instructions = out


def build():
    nc = bass.Bass()
    wj_in = nc.dram_tensor("wj", [KT, P, 2, WIN], fp8, kind="ExternalInput")
    xtc_in = nc.dram_tensor("xtc", [KT, P, RPC, 2], fp8, kind="ExternalInput")
    stat_in = nc.dram_tensor("stat", [P, NT + 1], f32, kind="ExternalInput")
    out_d = nc.dram_tensor("out", [RPC, WIN], bf16, kind="ExternalOutput")

    with tile.TileContext(nc) as tc:
        with (
            tc.tile_pool(name="xt", bufs=1) as xt_pool,
            tc.tile_pool(name="ot", bufs=1) as ot_pool,
            tc.tile_pool(name="small", bufs=1) as small_pool,
            tc.tile_pool(name="g", bufs=3, space="PSUM") as g_pool,
            tc.tile_pool(name="wup", bufs=1, space="PSUM") as wup_pool,
        ):
            # ---- input DMA first: xtc+wj[k] on sync/gpsimd, stat on scalar
            stat_sb = small_pool.tile([P, NT + 1], f32, tag="stat")
            nc.scalar.dma_start(stat_sb[:], stat_in[:, :])
            xtc = [xt_pool.tile([P, RPC, 2], fp8, name=f"xtc{k}", tag=f"xtc{k}")
                   for k in range(KT)]
            nc.sync.dma_start(xtc[0][:], xtc_in[0])
            nc.gpsimd.dma_start(xtc[1][:], xtc_in[1])
            wj = [xt_pool.tile([P, 2, WIN], fp8, name=f"wj{k}", tag=f"wj{k}")
                  for k in range(KT)]
            for w in range(W):
                sl = slice(w * JB, (w + 1) * JB)
                nc.sync.dma_start(wj[0][:, :, sl], wj_in[0, :, :, sl])
                nc.gpsimd.dma_start(wj[1][:, :, sl], wj_in[1, :, :, sl])

            # ---- one-time warmups ----------------------------------------
            # Exp ACT table load (~1.3us) during DMA dead time.
            warm_f = small_pool.tile([1, 16], f32, tag="warm_f")
            nc.vector.memset(warm_f[:], 1.0)
            warm_sb = small_pool.tile([1, 16], f32, tag="warm_sb")
            nc.scalar.activation(warm_sb[:], warm_f[:], ACTF.Exp)

            # PE p-state ramp: zeroed fp8 scratch, back-to-back DRS matmuls
            # into a scratch PSUM bank keep the Tensor engine continuously
            # busy from ~0.5us so real matmuls start near max clock.
            wstat = small_pool.tile([P, P, 2], fp8, tag="wstat")
            nc.vector.memset(wstat[:], 0.0)
            wmov = small_pool.tile([P, 2, P], fp8, tag="wmov")
            nc.vector.memset(wmov[:], 0.0)
            wp = wup_pool.tile([P, P], f32, tag="wup")
            for i in range(NWARM):
                nc.tensor.matmul(wp[:], wstat[:], wmov[:],
                                 start=True, stop=True, perf_mode=DRS)

            # ---- main GEMM + fused epilogue ------------------------------
            outq = [nc.sync, nc.gpsimd]
            qi = 0
            for off, width in GROUPS:
                for t in range(NT):
                    gp = g_pool.tile([P, width], f32, name=f"gp_{t}_{off}",
                                     tag="g")
                    for s in range(width // JB):
                        for k in range(KT):
                            nc.tensor.matmul(
                                gp[:, s * JB:(s + 1) * JB],
                                xtc[k][:, t * P:(t + 1) * P, :],
                                wj[k][:, :, off + s * JB:off + (s + 1) * JB],
                                start=(k == 0), stop=(k == KT - 1),
                                perf_mode=DRS)
                    ot = ot_pool.tile([P, width], bf16,
                                      name=f"ot_{t}_{off}", tag="ot", bufs=4)
                    nc.scalar.activation(ot[:], gp[:], ACTF.Exp,
                                         bias=stat_sb[:, t:t + 1],
                                         scale=stat_sb[:, NT:NT + 1])
                    outq[qi % 2].dma_start(
                        out_d[t * P:(t + 1) * P, off:off + width], ot[:])
                    qi += 1

    _split_waits(nc)
    return nc


_NC = None


def _dr_layout(a):
    """[512, M] -> [KT, P, 2, M] DoubleRow plane layout (d = kt*256+i*128+p)."""
    return np.ascontiguousarray(a.reshape(KT, 2, P, a.shape[1])
                                .transpose(0, 2, 1, 3))


def _swi_layout(a):
    """[512, M] -> [KT, P, M, 2] DoubleRowSwInterleave weights: per k-tile,
    (plane0, plane1) pairs adjacent along the last axis, with the m index
    reversed inside each 128-wide stationary slice. Logical dim 480
    (kt=1, p=96, plane=1) is the augmentation row: its weight is 1 and the
    host writes q_j into the matching wj slot."""
    m = a.shape[1]
    w = a.reshape(KT, 2, P, m).transpose(0, 2, 3, 1)    # [kt, p, m, i]
    w = w.reshape(KT, P, m // P, P, 2)[:, :, :, ::-1, :]
    w = np.ascontiguousarray(w.reshape(KT, P, m, 2))
    w[1, 96, :, 1] = 1.0
    return w


def make_in_maps(X):
    import ml_dtypes
    Xf = np.asarray(X, dtype=np.float64)
    X8 = np.asarray(X, dtype=ml_dtypes.float8_e4m3)
    XT8 = np.ascontiguousarray(X8.T)              # [512, 4096]

    # host-side stats: sq of the QUANTIZED points (what the GEMM computes),
    # sigma^2 of the ORIGINAL points (the reference's divisor).
    sq = (XT8.astype(np.float64) ** 2).sum(axis=0)          # [4096]
    mu = Xf.mean(axis=0)
    sigma2 = 2.0 * ((Xf ** 2).sum(axis=1).mean() - mu @ mu)  # mean(d2), exact
    q8 = np.asarray(-(sq - 512.0) / 2.0, dtype=ml_dtypes.float8_e4m3)
    scale = 1.0 / sigma2
    bias = -(sq + 512.0) / (2.0 * sigma2)                    # [4096]

    maps = []
    for c in range(NCORES):
        lo = c * RPC
        idx = (lo + np.arange(WIN)) % N
        wjc = _dr_layout(XT8[:, idx])
        wjc[1, 96, 1, :] = q8[idx]          # aug row: q_j rides the matmul
        stat = np.empty((P, NT + 1), dtype=np.float32)
        stat[:, :NT] = bias[lo:lo + RPC].reshape(NT, P).T
        stat[:, NT] = scale
        maps.append({
            "wj": wjc,
            "xtc": _swi_layout(XT8[:, lo:lo + RPC]),
            "stat": stat,
        })
    return maps


def assemble(slabs):
    """slabs: per-core [RPC, WIN] (bf16) -> full [N, N] f32 via symmetry."""
    out = np.empty((N, N), dtype=np.float32)
    for c in range(NCORES):
        lo = c * RPC
        slab = np.asarray(slabs[c], dtype=np.float32)
        n1 = min(WIN, N - lo)
        out[lo:lo + RPC, lo:lo + n1] = slab[:, :n1]
        if n1 < WIN:
            out[lo:lo + RPC, :WIN - n1] = slab[:, n1:]
    # mirror block-distance {5,6,7} from their transposed {3,2,1} partners
    for bi in range(NCORES):
        for dd in (5, 6, 7):
            bj = (bi + dd) % NCORES
            out[bi * RPC:(bi + 1) * RPC, bj * RPC:(bj + 1) * RPC] = \
                out[bj * RPC:(bj + 1) * RPC, bi * RPC:(bi + 1) * RPC].T
    return out


def kernel(X: np.ndarray) -> np.ndarray:
    global _NC
    if _NC is None:
        _NC = build()
    res = run_bass_kernel_spmd(_NC, make_in_maps(X),
                               list(range(NCORES))).results
    return assemble([res[c]["out"] for c in range(NCORES)])


# revision 10
# speedup vs baseline: 1.3831x; 1.1242x over previous
"""Gaussian kernel matrix on 8 Trainium2 NeuronCores — host-stats fp8 GEMM.

out = exp(-d2 / (2*sigma^2)),  d2[i,j] = ||x_i||^2 + ||x_j||^2 - 2 x_i.x_j,
sigma^2 = mean(d2) = 2*(mean(sq) - ||mean(X)||^2).

Strategy v2:
- Symmetry: core c computes rows [c*512,(c+1)*512) x a wrapped column window
  of 2560 cols starting at c*512 (5 of 8 j-blocks). Every unordered (i,j)
  pair is covered by at least one core; the host mirrors the remaining
  blocks by transposition. 0.625x compute/output vs full slabs.
- GEMM in fp8 e4m3 with DoubleRowSwInterleave (K=256 per matmul): X
  quantized on host; the kernel computes the EXACT Gaussian kernel of the
  quantized points (fro err ~3e-3 incl bf16 output, gate is 2e-2).
- ALL statistics on the host (O(N*D), trivial next to the O(N^2*D) GEMM):
  sq_j, sigma^2 (exact, from unquantized X), q_j = -(sq_j-512)/2 written
  into the sacrificed contraction row (logical dim 480) of wj so the
  column term rides the main matmul; bias_i/scale shipped as a tiny [P,5]
  f32 input consumed directly by the fused ACT epilogue
  out = Exp(scale*G + bias_i) -> bf16 -> DMA. The device runs NOTHING but
  DMA + matmul + activation.
- PE p-state: the Tensor engine only reaches max clock after ~3us of
  continuous busy and drops back on idle gaps. Dummy warmup matmuls on a
  zeroed scratch tile start the ramp during the input-DMA dead time and
  bridge until the first wj chunk lands.
- ACT overhead amortized by pairing j-blocks: PSUM tiles span 2 banks
  [128,1024] and one Exp pass covers both, halving the ~250ns fixed
  PSUM-access cost per instruction.
- DMA spread across sync/gpsimd/vector queues (inputs and outputs) so no
  single queue serializes; outputs rotate across the three.
"""
import numpy as np
import sys

sys.path.insert(0, "/opt/trn_rl_repo")
from concourse import bass, tile, mybir  # noqa: E402
from concourse.bass_utils import run_bass_kernel_spmd  # noqa: E402

N, D, NCORES = 4096, 512, 8
RPC = 512                  # output rows per core
P = 128                    # partitions
KT = 2                     # DoubleRow k-tiles (256 contraction rows each)
JB = 512                   # j-block width
W = 5                      # window j-blocks per core
WIN = W * JB               # 2560 window columns
NT = RPC // P              # 4 row-tiles per core
NWARM = 20                 # PE warmup matmuls (~215ns each)
f32 = mybir.dt.float32
bf16 = mybir.dt.bfloat16
fp8 = mybir.dt.float8e4
ACTF = mybir.ActivationFunctionType
DRS = mybir.MatmulPerfMode.DoubleRowSwInterleave

# (window col offset, width) groups: narrow group first so the first Exp
# starts as soon as possible (input DMA still landing); later groups pair
# j-blocks into 2-bank PSUM tiles for wide ACT passes.
GROUPS = [(0, 512), (512, 1024), (1536, 1024)]


def _split_waits(nc, max_waits=1):
    """walrus in this image encodes at most one sync-wait per instruction;
    split extras into single-wait NOPs placed just before the instruction."""
    for fn in nc.m.functions:
        for bb in fn.blocks:
            out = []
            for inst in bb.instructions:
                si = inst.sync_info
                if si and si.on_wait and len(si.on_wait) > max_waits:
                    waits = list(si.on_wait)
                    extra, keep = waits[:-max_waits], waits[-max_waits:]
                    for j, w in enumerate(extra):
                        out.append(mybir.InstNoOp(
                            name=f"{inst.name}-ws{j}", engine=inst.engine,
                            sync_info=mybir.SyncInfo(on_wait=[w], on_update=[])))
                    si.on_wait = keep
                out.append(inst)
            bb.instructions = out


def build():
    nc = bass.Bass()
    wj_in = nc.dram_tensor("wj", [KT, P, 2, WIN], fp8, kind="ExternalInput")
    xtc_in = nc.dram_tensor("xtc", [KT, P, RPC, 2], fp8, kind="ExternalInput")
    stat_in = nc.dram_tensor("stat", [P, NT + 1], f32, kind="ExternalInput")
    out_d = nc.dram_tensor("out", [RPC, WIN], bf16, kind="ExternalOutput")

    with tile.TileContext(nc) as tc:
        with (
            tc.tile_pool(name="xt", bufs=1) as xt_pool,
            tc.tile_pool(name="ot", bufs=1) as ot_pool,
            tc.tile_pool(name="small", bufs=1) as small_pool,
            tc.tile_pool(name="g", bufs=3, space="PSUM") as g_pool,
            tc.tile_pool(name="wup", bufs=1, space="PSUM") as wup_pool,
        ):
            # ---- input DMA first --------------------------------------
            # Critical path: first matmul needs xtc0+wj[0]w0 (k=0) then
            # xtc1+wj[1]w0 (k=1); DMA completion semaphores cost ~0.9us,
            # so the first-needed chunks go FIRST on each of the three
            # DMA-capable queues (sync/SP, gpsimd, scalar).
            xtc = [xt_pool.tile([P, RPC, 2], fp8, name=f"xtc{k}", tag=f"xtc{k}")
                   for k in range(KT)]
            wj = [xt_pool.tile([P, 2, WIN], fp8, name=f"wj{k}", tag=f"wj{k}")
                  for k in range(KT)]
            stat_sb = small_pool.tile([P, NT + 1], f32, tag="stat")
            sl0 = slice(0, JB)
            nc.scalar.dma_start(wj[0][:, :, sl0], wj_in[0, :, :, sl0])
            nc.scalar.dma_start(stat_sb[:], stat_in[:, :])
            nc.sync.dma_start(xtc[0][:], xtc_in[0])
            nc.sync.dma_start(xtc[1][:], xtc_in[1])
            nc.gpsimd.dma_start(wj[1][:, :, sl0], wj_in[1, :, :, sl0])
            for w in range(1, W):
                sl = slice(w * JB, (w + 1) * JB)
                nc.sync.dma_start(wj[0][:, :, sl], wj_in[0, :, :, sl])
                nc.gpsimd.dma_start(wj[1][:, :, sl], wj_in[1, :, :, sl])

            # Exp ACT table load (~1.3us) during DMA dead time.
            warm_f = small_pool.tile([1, 16], f32, tag="warm_f")
            nc.vector.memset(warm_f[:], 1.0)
            warm_sb = small_pool.tile([1, 16], f32, tag="warm_sb")
            nc.scalar.activation(warm_sb[:], warm_f[:], ACTF.Exp)

            # PE p-state ramp: the Tensor engine reaches max clock only
            # after ~3us of continuous busy and any idle gap resets it.
            # Back-to-back DRS matmuls over raw (uninitialized) SBUF — no
            # memset, no tile deps — start the ramp right after the PE
            # preamble and bridge until the first wj chunk lands. Garbage
            # values land in a scratch PSUM bank and are never read.
            wstat = nc.alloc_sbuf_tensor("wup_stat", [P, P, 2], fp8).ap()
            wmov = nc.alloc_sbuf_tensor("wup_mov", [P, 2, 256], fp8).ap()
            wp = wup_pool.tile([P, 256], f32, tag="wup")
            for i in range(NWARM):
                nc.tensor.matmul(wp[:], wstat, wmov,
                                 start=True, stop=True, perf_mode=DRS)

            # ---- main GEMM + fused epilogue ------------------------------
            outq = [nc.gpsimd, nc.sync]
            qi = 0
            for off, width in GROUPS:
                for t in range(NT):
                    gp = g_pool.tile([P, width], f32, name=f"gp_{t}_{off}",
                                     tag="g")
                    for s in range(width // JB):
                        for k in range(KT):
                            nc.tensor.matmul(
                                gp[:, s * JB:(s + 1) * JB],
                                xtc[k][:, t * P:(t + 1) * P, :],
                                wj[k][:, :, off + s * JB:off + (s + 1) * JB],
                                start=(k == 0), stop=(k == KT - 1),
                                perf_mode=DRS)
                    ot = ot_pool.tile([P, width], bf16,
                                      name=f"ot_{t}_{off}", tag="ot", bufs=8)
                    nc.scalar.activation(ot[:], gp[:], ACTF.Exp,
                                         bias=stat_sb[:, t:t + 1],
                                         scale=stat_sb[:, NT:NT + 1])
                    outq[qi % 2].dma_start(
                        out_d[t * P:(t + 1) * P, off:off + width], ot[:])
                    qi += 1

    _split_waits(nc)
    return nc


_NC = None


def _dr_layout(a):
    """[512, M] -> [KT, P, 2, M] DoubleRow plane layout (d = kt*256+i*128+p)."""
    return np.ascontiguousarray(a.reshape(KT, 2, P, a.shape[1])
                                .transpose(0, 2, 1, 3))


def _swi_layout(a):
    """[512, M] -> [KT, P, M, 2] DoubleRowSwInterleave weights: per k-tile,
    (plane0, plane1) pairs adjacent along the last axis, with the m index
    reversed inside each 128-wide stationary slice. Logical dim 480
    (kt=1, p=96, plane=1) is the augmentation row: its weight is 1 and the
    host writes q_j into the matching wj slot."""
    m = a.shape[1]
    w = a.reshape(KT, 2, P, m).transpose(0, 2, 3, 1)    # [kt, p, m, i]
    w = w.reshape(KT, P, m // P, P, 2)[:, :, :, ::-1, :]
    w = np.ascontiguousarray(w.reshape(KT, P, m, 2))
    w[1, 96, :, 1] = 1.0
    return w


def make_in_maps(X):
    import ml_dtypes
    Xf = np.asarray(X, dtype=np.float64)
    X8 = np.asarray(X, dtype=ml_dtypes.float8_e4m3)
    XT8 = np.ascontiguousarray(X8.T)              # [512, 4096]

    # host-side stats: sq of the QUANTIZED points (what the GEMM computes),
    # sigma^2 of the ORIGINAL points (the reference's divisor).
    sq = (XT8.astype(np.float64) ** 2).sum(axis=0)          # [4096]
    mu = Xf.mean(axis=0)
    sigma2 = 2.0 * ((Xf ** 2).sum(axis=1).mean() - mu @ mu)  # mean(d2), exact
    q8 = np.asarray(-(sq - 512.0) / 2.0, dtype=ml_dtypes.float8_e4m3)
    scale = 1.0 / sigma2
    bias = -(sq + 512.0) / (2.0 * sigma2)                    # [4096]

    maps = []
    for c in range(NCORES):
        lo = c * RPC
        idx = (lo + np.arange(WIN)) % N
        wjc = _dr_layout(XT8[:, idx])
        wjc[1, 96, 1, :] = q8[idx]          # aug row: q_j rides the matmul
        stat = np.empty((P, NT + 1), dtype=np.float32)
        stat[:, :NT] = bias[lo:lo + RPC].reshape(NT, P).T
        stat[:, NT] = scale
        maps.append({
            "wj": wjc,
            "xtc": _swi_layout(XT8[:, lo:lo + RPC]),
            "stat": stat,
        })
    return maps


def assemble(slabs):
    """slabs: per-core [RPC, WIN] (bf16) -> full [N, N] f32 via symmetry."""
    out = np.empty((N, N), dtype=np.float32)
    for c in range(NCORES):
        lo = c * RPC
        slab = np.asarray(slabs[c], dtype=np.float32)
        n1 = min(WIN, N - lo)
        out[lo:lo + RPC, lo:lo + n1] = slab[:, :n1]
        if n1 < WIN:
            out[lo:lo + RPC, :WIN - n1] = slab[:, n1:]
    # mirror block-distance {5,6,7} from their transposed {3,2,1} partners
    for bi in range(NCORES):
        for dd in (5, 6, 7):
            bj = (bi + dd) % NCORES
            out[bi * RPC:(bi + 1) * RPC, bj * RPC:(bj + 1) * RPC] = \
                out[bj * RPC:(bj + 1) * RPC, bi * RPC:(bi + 1) * RPC].T
    return out


def kernel(X: np.ndarray) -> np.ndarray:
    global _NC
    if _NC is None:
        _NC = build()
    res = run_bass_kernel_spmd(_NC, make_in_maps(X),
                               list(range(NCORES))).results
    return assemble([res[c]["out"] for c in range(NCORES)])
